# revision 2
# baseline (speedup 1.0000x reference)
"""Balanced grouped-expert SwiGLU kernel (v5: S=512 slots).

Per-matmul cost on this toolchain is (stream + ~128c LDWEIGHTS), measured
265 ns at free=512 / 145 ns at free=264 — the serial weight load favors
the widest legal moving operand (512, the f32 PSUM bank limit), which
beats the finer 264-token tiling despite 9% more padded tokens.

v2 -> v3:
- All pool tile allocations hoisted OUT of the If/Else branches (allocation
  is scheduler bookkeeping; per-slot instead of per-branch keeps rotation
  and cross-slot dependencies precise).
- Output DMAs issued per d-group (4/slot, 540KB each) on the GpSimd queue
  (SP = inputs, Act = silu only, GpSimd = outputs).
- Slot 0 runs k-outer (all 4 f-chunk PSUM pairs live) so first matmuls need
  only the first k-half of wa1/wa3 and xt0; those DMAs are split in halves
  and ordered first.
"""

import math
import os

import ml_dtypes
import numpy as np

D = 2048
F = 512
S = 512
KC = D // 128
FC = F // 128
DC = D // 128
NCORES = 8

_cache = {}


def _build(nt: int, s: int = S, loop: bool = False):
    import concourse.bacc as bacc
    import concourse.mybir as mybir
    from concourse.tile import TileContext

    dt = mybir.dt
    f32 = dt.float32
    bf16 = dt.bfloat16
    i32 = dt.int32
    PAD_T = nt * s

    nc = bacc.Bacc(
        "TRN2", target_bir_lowering=False, debug=False,
        enable_asserts=False, num_devices=NCORES,
    )

    xpt = nc.dram_tensor("xpt", [D, PAD_T], bf16, kind="ExternalInput")
    wa1 = nc.dram_tensor("wa1", [D, F], bf16, kind="ExternalInput")
    wa2 = nc.dram_tensor("wa2", [F, D], bf16, kind="ExternalInput")
    wa3 = nc.dram_tensor("wa3", [D, F], bf16, kind="ExternalInput")
    wb1 = nc.dram_tensor("wb1", [D, F], bf16, kind="ExternalInput")
    wb2 = nc.dram_tensor("wb2", [F, D], bf16, kind="ExternalInput")
    wb3 = nc.dram_tensor("wb3", [D, F], bf16, kind="ExternalInput")
    meta = nc.dram_tensor("meta", [1, 2], i32, kind="ExternalInput")
    outT = nc.dram_tensor("out", [D, PAD_T], f32, kind="ExternalOutput")

    xpt_r = xpt.ap().rearrange("(k p) t -> p k t", p=128)
    out_r = outT.ap().rearrange("(c p) t -> p c t", p=128)
    wa1_r = wa1.ap().rearrange("(k p) f -> p k f", p=128)
    wa3_r = wa3.ap().rearrange("(k p) f -> p k f", p=128)

    with TileContext(nc) as tc:
        with (
            tc.tile_pool(name="wpool", bufs=1) as wpool,
            tc.tile_pool(name="xt", bufs=3) as xt_pool,
            tc.tile_pool(name="ht", bufs=2) as ht_pool,
            tc.tile_pool(name="sil", bufs=4) as sil_pool,
            tc.tile_pool(name="osb", bufs=3) as osb_pool,
            tc.tile_pool(name="psx", bufs=4, space="PSUM") as psx_pool,
            tc.tile_pool(name="pso", bufs=4, space="PSUM") as pso_pool,
        ):
            msb = wpool.tile([1, 2], i32, tag="meta")
            nc.sync.dma_start(out=msb[:], in_=meta.ap())

            xts = {}

            def emit_xt(m):
                xt = xt_pool.tile([128, KC, s], bf16, tag="xt")
                nc.sync.dma_start(
                    out=xt[:], in_=xpt_r[:, :, m * s:(m + 1) * s])
                xts[m] = xt

            wsb = {}
            w_dram = {"a": (wa1, wa2, wa3), "b": (wb1, wb2, wb3)}
            for pre in ("a", "b"):
                s1 = wpool.tile([128, KC, F], bf16, tag=f"w1{pre}")
                s3 = wpool.tile([128, KC, F], bf16, tag=f"w3{pre}")
                s2 = wpool.tile([128, FC, D], bf16, tag=f"w2{pre}")
                wsb[pre] = (s1, s2, s3)

            wa2_r = wa2.ap().rearrange("(c p) d -> p c d", p=128)

            def emit_startup():
                # SP order: first k-quarters of xt0/wa1 (slot-0 x1 needs),
                # then wa3 halves, then the rest of the weights.
                xt0 = xt_pool.tile([128, KC, s], bf16, tag="xt", name="xt0")
                xts[0] = xt0
                Q = KC // 4
                H = KC // 2
                for q in range(4):
                    nc.sync.dma_start(out=xt0[:, q * Q:(q + 1) * Q, :],
                                      in_=xpt_r[:, q * Q:(q + 1) * Q, 0:s])
                    nc.sync.dma_start(out=wsb["a"][0][:, q * Q:(q + 1) * Q, :],
                                      in_=wa1_r[:, q * Q:(q + 1) * Q, :])
                nc.sync.dma_start(out=wsb["a"][2][:, 0:H, :],
                                  in_=wa3_r[:, 0:H, :])
                nc.sync.dma_start(out=wsb["a"][2][:, H:KC, :],
                                  in_=wa3_r[:, H:KC, :])
                nc.sync.dma_start(
                    out=wsb["a"][1][:, :, 0:512], in_=wa2_r[:, :, 0:512])
                if nt > 1:
                    emit_xt(1)
                for g in range(1, 4):
                    nc.sync.dma_start(
                        out=wsb["a"][1][:, :, g * 512:(g + 1) * 512],
                        in_=wa2_r[:, :, g * 512:(g + 1) * 512])
                nc.sync.dma_start(
                    out=wsb["b"][0][:],
                    in_=wb1.ap().rearrange("(k p) f -> p k f", p=128))
                nc.sync.dma_start(
                    out=wsb["b"][2][:],
                    in_=wb3.ap().rearrange("(k p) f -> p k f", p=128))
                nc.sync.dma_start(
                    out=wsb["b"][1][:],
                    in_=wb2.ap().rearrange("(c p) d -> p c d", p=128))

            ta_v = nc.snap(nc.values_load(msb[0:1, 0:1]))

            def alloc_slot_tiles(kouter):
                tl = {}
                tl["ht"] = ht_pool.tile([128, FC, s], bf16, tag="ht", name="ht")
                if kouter:
                    pairs = []
                    for f in range(FC):
                        x1t = psx_pool.tile([128, 512], f32, tag="psx", name="x1t")
                        x3t = pso_pool.tile([128, 512], f32, tag="pso", name="x3t")
                        pairs.append((x1t, x3t))
                    tl["pairs"] = pairs
                else:
                    pairs = []
                    for f in range(FC):
                        x1t = psx_pool.tile([128, 512], f32, tag="psx", name="x1t")
                        x3t = psx_pool.tile([128, 512], f32, tag="psx", name="x3t")
                        pairs.append((x1t, x3t))
                    tl["pairs"] = pairs
                sils = []
                for f in range(FC):
                    sil = sil_pool.tile([128, s], f32, tag="sil", name="sil")
                    sils.append(sil)
                tl["sils"] = sils
                gs = []
                for g in range(4):
                    pos = []
                    for _ in range(4):
                        po = pso_pool.tile([128, 512], f32, tag="pso", name="po")
                        pos.append(po)
                    osb = osb_pool.tile([128, 4, s], f32, tag="osb", name="osb")
                    gs.append((pos, osb))
                tl["gs"] = gs
                return tl

            def gate_mul(m, tl, f):
                sil = tl["sils"][f]
                x1t, x3t = tl["pairs"][f]
                nc.scalar.activation(
                    sil[:], x1t[:, :s], mybir.ActivationFunctionType.Silu)
                nc.vector.tensor_mul(tl["ht"][:, f, :], sil[:], x3t[:, :s])

            def w2_phase(m, tl, w2_sb):
                ht = tl["ht"]
                for g in range(4):
                    pos, osb = tl["gs"][g]
                    for fc in range(FC):
                        for j in range(4):
                            d = g * 4 + j
                            nc.tensor.matmul(
                                pos[j][:, :s],
                                w2_sb[:, fc, d * 128:(d + 1) * 128],
                                ht[:, fc, :],
                                start=(fc == 0), stop=(fc == FC - 1))
                    for j in range(4):
                        nc.vector.tensor_copy(osb[:, j, :], pos[j][:, :s])
                    nc.scalar.dma_start(
                        out=out_r[:, g * 4:(g + 1) * 4, m * s:(m + 1) * s],
                        in_=osb[:])

            def body(m, which):
                tl = alloc_slot_tiles(kouter=False)
                w1_sb, w2_sb, w3_sb = wsb[which]
                xt = xts[m]
                for f in range(FC):
                    x1t, x3t = tl["pairs"][f]
                    for k in range(KC):
                        nc.tensor.matmul(
                            x1t[:, :s], w1_sb[:, k, f * 128:(f + 1) * 128],
                            xt[:, k, :], start=(k == 0), stop=(k == KC - 1))
                        nc.tensor.matmul(
                            x3t[:, :s], w3_sb[:, k, f * 128:(f + 1) * 128],
                            xt[:, k, :], start=(k == 0), stop=(k == KC - 1))
                    gate_mul(m, tl, f)
                w2_phase(m, tl, w2_sb)

            def body_kouter(m, which):
                tl = alloc_slot_tiles(kouter=True)
                w1_sb, w2_sb, w3_sb = wsb[which]
                xt = xts[m]
                for k in range(KC):
                    for f in range(FC):
                        x1t, _ = tl["pairs"][f]
                        nc.tensor.matmul(
                            x1t[:, :s], w1_sb[:, k, f * 128:(f + 1) * 128],
                            xt[:, k, :], start=(k == 0), stop=(k == KC - 1))
                for k in range(KC):
                    for f in range(FC):
                        _, x3t = tl["pairs"][f]
                        nc.tensor.matmul(
                            x3t[:, :s], w3_sb[:, k, f * 128:(f + 1) * 128],
                            xt[:, k, :], start=(k == 0), stop=(k == KC - 1))
                for f in range(FC):
                    gate_mul(m, tl, f)
                w2_phase(m, tl, w2_sb)

            def emit_all():
                emit_startup()
                for m in range(nt):
                    if m + 2 < nt:
                        emit_xt(m + 2)
                    emit = body_kouter if m == 0 else body
                    with tc.If(ta_v > m) as cmp:
                        emit(m, "a")
                    with cmp.Else():
                        emit(m, "b")

            if loop:
                nreps_v = nc.snap(nc.values_load(msb[0:1, 1:2]))
                with tc.For_i(0, nreps_v):
                    emit_all()
            else:
                emit_all()

    nc.compile()
    return nc


def _get_program(nt: int):
    if nt not in _cache:
        _cache[nt] = _build(nt)
    return _cache[nt]


def _assign(counts, s, ncap=None):
    """Greedy: chunk the padded-tile list into per-core runs of <=cap tiles
    spanning <=2 experts. Returns (nt, per-core list of (expert, tile_lo,
    n_tiles) segments) or None if infeasible."""
    E = len(counts)
    pt = [max(1, math.ceil(c / s)) if c > 0 else 0 for c in counts]
    total = sum(pt)
    nt = ncap if ncap is not None else math.ceil(total / NCORES)
    for nt_try in (nt, nt + 1, nt + 2):
        segs = [[] for _ in range(NCORES)]
        e, used = 0, 0
        for c in range(NCORES):
            cap = nt_try
            nexp = 0
            while cap > 0 and e < E:
                if pt[e] - used == 0:
                    e += 1
                    used = 0
                    continue
                if nexp == 2:
                    break
                take = min(cap, pt[e] - used)
                segs[c].append((e, used, take))
                used += take
                cap -= take
                nexp += 1
        leftover = total - sum(sg[2] for core in segs for sg in core)
        if leftover == 0:
            return nt_try, segs
    return None


def kernel(x, num_tokens_per_expert, w1, w2, w3):
    from concourse.bass_utils import run_bass_kernel_spmd

    x = np.asarray(x)
    counts = [int(v) for v in np.asarray(num_tokens_per_expert)]
    w1 = np.asarray(w1)
    w2 = np.asarray(w2)
    w3 = np.asarray(w3)
    T, E = x.shape[0], len(counts)
    assert E == NCORES
    starts = np.concatenate([[0], np.cumsum(counts)])[:E].astype(np.int64)

    plan = _assign(counts, S)
    if plan is None:
        # fallback: expert-parallel (1 segment per core), padded to max tiles
        pt = [max(1, math.ceil(c / S)) if c > 0 else 0 for c in counts]
        nt = max(pt)
        segs = [[(e, 0, pt[e])] if pt[e] else [] for e in range(NCORES)]
        plan = (nt, segs)
    nt, segs = plan
    nc = _get_program(nt)
    PAD_T = nt * S

    w1b = w1.astype(ml_dtypes.bfloat16)
    w2b = w2.astype(ml_dtypes.bfloat16)
    w3b = w3.astype(ml_dtypes.bfloat16)
    xT = np.ascontiguousarray(x.T).astype(ml_dtypes.bfloat16)  # [D, T]

    in_maps = []
    placements = []  # per core: list of (slot, src_lo, n_rows)
    for c in range(NCORES):
        xpt = np.zeros((D, PAD_T), dtype=ml_dtypes.bfloat16)
        place = []
        slot = 0
        cs = segs[c]
        ta = cs[0][2] if cs else 0
        exps = [sg[0] for sg in cs]
        ea = exps[0] if exps else 0
        eb = exps[1] if len(exps) > 1 else ea
        for (e, tile_lo, ntk) in cs:
            src_lo = int(starts[e]) + tile_lo * S
            src_hi = min(int(starts[e]) + counts[e], src_lo + ntk * S)
            nrow = src_hi - src_lo
            xpt[:, slot * S: slot * S + nrow] = xT[:, src_lo:src_hi]
            place.append((slot, src_lo, nrow))
            slot += ntk
        placements.append(place)
        in_maps.append({
            "xpt": xpt,
            "wa1": np.ascontiguousarray(w1b[ea]),
            "wa2": np.ascontiguousarray(w2b[ea]),
            "wa3": np.ascontiguousarray(w3b[ea]),
            "wb1": np.ascontiguousarray(w1b[eb]),
            "wb2": np.ascontiguousarray(w2b[eb]),
            "wb3": np.ascontiguousarray(w3b[eb]),
            "meta": np.array([[ta, 1]], dtype=np.int32),
        })

    trace = bool(int(os.environ.get("KERNEL_TRACE", "0")))
    try:
        res = run_bass_kernel_spmd(nc, in_maps, core_ids=list(range(NCORES)),
                                   trace=trace)
    except ModuleNotFoundError:
        res = run_bass_kernel_spmd(nc, in_maps, core_ids=list(range(NCORES)),
                                   trace=False)
    kernel.last_results = res

    out = np.empty((T, D), dtype=np.float32)
    for c in range(NCORES):
        o = res.results[c]["out"]  # [D, PAD_T] transposed
        for (slot, src_lo, nrow) in placements[c]:
            out[src_lo:src_lo + nrow] = o[:, slot * S: slot * S + nrow].T
    return out
